# revision 7
# baseline (speedup 1.0000x reference)
"""Gated axial attention (width axis) Trainium2 Bass kernel.

Sharding: data-parallel over the fused B*H row axis (512 rows -> 64 rows
per core on 8 cores). Each core computes full attention for its rows;
no collectives. All matmuls run in bf16 with fp32 PSUM accumulation.

v2 design (vs baseline):
- Scores in ONE K=128 matmul per (head, row) instead of three K=64
  matmuls: contract [k + gq*pq ; k] (kk) against [q ; gk*s*pk] (qq),
  both stacked to 128 partitions per head.  kk native-parity halves are
  built by GPSIMD adds (k + pq_rep); opposite halves and the qq q-halves
  are built by SBUF->SBUF DMA from the projection outputs; qq pk-halves
  are constant (DMA'd once from DRAM).
- All score/AV matmul operands sit at full 128 partitions (no even/odd
  parity PSUM split) -> simple [128,512] score PSUM, double-buffered.
- Softmax normalize uses one broadcast tensor_mul per (row, half) with a
  stride-0 AP on the reciprocal denominators (replaces 8 tiny
  tensor_scalar ops per row).
- Per-row PE emission interleaves scores / V-proj(r+1) / transpose(r-1) /
  AV so the Exp ACT latency is hidden without double PSUM.
- Last-4-rows transposes + o-projection of block b are emitted inside
  block b+1's projection phase so the DVE backlog drains off-critical.
- Startup const DMAs spread across gpsimd/vector/scalar queues.

Scale folding (host side): 1/sqrt(hd)=0.125 into q_w/q_b; g_q into
pq_rep; g_k*0.125 into the qq pk-halves; g_v1 into v_w and v_b; g_v2
into pos_v; o_b added on host after gathering.
"""

import sys
import types

sys.path.insert(0, "/opt/trn_rl_repo")


def _install_ntff_shim():
    """Make bass_utils trace=True work under axon (BASS_TRACE=1)."""
    try:
        import antenv
    except ImportError:
        return
    if "antenv.axon_hooks" in sys.modules:
        return
    mod = types.ModuleType("antenv.axon_hooks")
    _hook = [None]

    def set_axon_ntff_profile_hook(h):
        _hook[0] = h

    def get_axon_ntff_profile_hook():
        if _hook[0] is None:
            try:
                if "/root/.axon_site" not in sys.path:
                    sys.path.insert(0, "/root/.axon_site")
                from trn_agent_boot.trn_boot import _ntff_profile_via_ctypes

                _hook[0] = _ntff_profile_via_ctypes("/opt/axon/libaxon_pjrt.so")
            except Exception:
                _hook[0] = None
        return _hook[0]

    mod.set_axon_ntff_profile_hook = set_axon_ntff_profile_hook
    mod.get_axon_ntff_profile_hook = get_axon_ntff_profile_hook
    sys.modules["antenv.axon_hooks"] = mod
    antenv.axon_hooks = mod


_install_ntff_shim()

import ml_dtypes  # noqa: E402
import numpy as np  # noqa: E402

import concourse.bass as bass  # noqa: E402
import concourse.tile as tile  # noqa: E402
from concourse import bacc, mybir  # noqa: E402
from concourse.bass_utils import run_bass_kernel_spmd  # noqa: E402

BF16 = ml_dtypes.bfloat16

B, C, H, W = 4, 512, 128, 128
NH, HD = 8, 64
NCORES = 8
ROWS = B * H  # 512 fused rows
RPC = ROWS // NCORES  # 64 rows per core
BLK = 8  # rows per block
NBLK = RPC // BLK
P = 128
NCH = C // P  # 4 channel chunks of 128
TOK = BLK * W  # tokens per block (1024)
NTT = TOK // 512  # 512-token tiles per block (2)

_CACHED_NC = None
LAST_RESULTS = None


def _build_nc():
    nc = bacc.Bacc("TRN2", target_bir_lowering=False, debug=False,
                   num_devices=NCORES)
    dt = mybir.dt

    xt = nc.dram_tensor("xt", [NCH, P, RPC, W], dt.bfloat16,
                        kind="ExternalInput")
    q_wt = nc.dram_tensor("q_wt", [C, C], dt.bfloat16, kind="ExternalInput")
    k_wt = nc.dram_tensor("k_wt", [C, C], dt.bfloat16, kind="ExternalInput")
    v_wt = nc.dram_tensor("v_wt", [C, C], dt.bfloat16, kind="ExternalInput")
    o_wt = nc.dram_tensor("o_wt", [C, C], dt.bfloat16, kind="ExternalInput")
    qb = nc.dram_tensor("qb", [C], dt.float32, kind="ExternalInput")
    kb = nc.dram_tensor("kb", [C], dt.float32, kind="ExternalInput")
    pq_rep_d = nc.dram_tensor("pq_rep", [NCH, P, TOK], dt.bfloat16,
                              kind="ExternalInput")
    qq0_d = nc.dram_tensor("qq0", [P, 2, NCH, TOK], dt.bfloat16,
                           kind="ExternalInput")
    pvs = nc.dram_tensor("pvs", [W, C], dt.bfloat16, kind="ExternalInput")
    ident = nc.dram_tensor("ident", [P, P], dt.bfloat16, kind="ExternalInput")
    out_t = nc.dram_tensor("out_t", [NCH, P, RPC, W], dt.float32,
                           kind="ExternalOutput")

    AF = mybir.ActivationFunctionType

    with tile.TileContext(nc) as tc:
        with (
            tc.tile_pool(name="const", bufs=1) as const,
            tc.tile_pool(name="xtp", bufs=2) as xtp,
            tc.tile_pool(name="qtp", bufs=1) as qtp,
            tc.tile_pool(name="ktp", bufs=1) as ktp,
            tc.tile_pool(name="vmixp", bufs=1) as vmixp,
            tc.tile_pool(name="expp", bufs=1) as expp,
            tc.tile_pool(name="aop", bufs=2) as aop,
            tc.tile_pool(name="aotp", bufs=2) as aotp,
            tc.tile_pool(name="small", bufs=4) as small,
            tc.tile_pool(name="fop", bufs=2) as fop,
            tc.tile_pool(name="ps_pp", bufs=2, space="PSUM") as ps_pp,
            tc.tile_pool(name="ps_sc", bufs=2, space="PSUM") as ps_sc,
            tc.tile_pool(name="ps_av", bufs=2, space="PSUM") as ps_av,
            tc.tile_pool(name="ps_tr", bufs=2, space="PSUM") as ps_tr,
        ):
            # ---- constants into SBUF (spread across DMA queues) ----
            def load_w(name, dram, eng):
                t = const.tile([P, NCH, C], dt.bfloat16, name=name)
                src = dram.ap().rearrange("(k p) c -> p k c", p=P)
                eng.dma_start(out=t, in_=src)
                return t

            # K-proj runs first: kw on gpsimd queue, first in line.
            kw_sb = load_w("kw_sb", k_wt, nc.gpsimd)
            qw_sb = load_w("qw_sb", q_wt, nc.gpsimd)
            vw_sb = load_w("vw_sb", v_wt, nc.scalar)
            ow_sb = load_w("ow_sb", o_wt, nc.scalar)

            pv_sb = const.tile([P, C], dt.bfloat16, name="pv_sb")
            nc.scalar.dma_start(out=pv_sb, in_=pvs.ap())
            id_sb = const.tile([P, P], dt.bfloat16, name="id_sb")
            nc.scalar.dma_start(out=id_sb, in_=ident.ap())
            qb_sb = const.tile([P, NCH], dt.float32, name="qb_sb")
            nc.scalar.dma_start(out=qb_sb,
                                in_=qb.ap().rearrange("(m p) -> p m", p=P))
            kb_sb = const.tile([P, NCH], dt.float32, name="kb_sb")
            nc.scalar.dma_start(out=kb_sb,
                                in_=kb.ap().rearrange("(m p) -> p m", p=P))
            pq_rep = const.tile([P, NCH, TOK], dt.bfloat16, name="pq_rep")
            nc.scalar.dma_start(out=pq_rep,
                                in_=pq_rep_d.ap().rearrange("k p t -> p k t"))
            # Persistent double-buffered qq/kk (manual A/B alternation).
            qq_ab = [const.tile([P, 2, NCH, TOK], dt.bfloat16, name="qqA"),
                     const.tile([P, 2, NCH, TOK], dt.bfloat16, name="qqB")]
            kk_ab = [const.tile([P, 2, NCH, TOK], dt.bfloat16, name="kkA"),
                     const.tile([P, 2, NCH, TOK], dt.bfloat16, name="kkB")]
            for t in qq_ab:
                nc.scalar.dma_start(out=t, in_=qq0_d.ap())

            xt_r = xt.ap()  # [NCH, P, RPC, W]
            out_r = out_t.ap()

            # ---- helpers ----
            def proj(wsb, m, n, xt_sb):
                ps = ps_pp.tile([P, 512], dt.float32, tag="pp", name="ps")
                for k in range(NCH):
                    nc.tensor.matmul(
                        ps,
                        lhsT=wsb[:, k, m * P:(m + 1) * P],
                        rhs=xt_sb[:, k, n * 512:(n + 1) * 512],
                        start=(k == 0), stop=(k == NCH - 1))
                return ps

            tr_state = {}  # parity -> pst tile awaiting its pair

            def emit_tr(r, ao_sb, aot_sb):
                # rows are transposed in pairs sharing one PSUM tile; the
                # copyback happens once per pair (on the odd row)
                if r % 2 == 0:
                    pst = ps_tr.tile([P, 2, NCH, P], dt.bfloat16, tag="tr",
                                     name="pst")
                    tr_state['pst'] = pst
                else:
                    pst = tr_state['pst']
                for ch in range(NCH):
                    nc.tensor.transpose(
                        pst[:, r % 2, ch, :],
                        ao_sb[:, r, ch * P:(ch + 1) * P], id_sb)
                if r % 2 == 1:
                    r0 = r - 1
                    nc.scalar.copy(
                        aot_sb[:, :, r0 * P:(r0 + 2) * P]
                        .rearrange("p k (r w) -> p r k w", r=2),
                        pst.rearrange("p r k w -> p r k w"))

            def emit_oproj_m(n, m, aot_sb, fo, eng):
                ps = ps_pp.tile([P, 512], dt.float32, tag="pp", name="ps")
                for k in range(NCH):
                    nc.tensor.matmul(
                        ps,
                        lhsT=ow_sb[:, k, m * P:(m + 1) * P],
                        rhs=aot_sb[:, k, n * 512:(n + 1) * 512],
                        start=(k == 0), stop=(k == NCH - 1))
                if eng is nc.scalar:
                    eng.copy(fo[:, m, :], ps)
                else:
                    eng.tensor_copy(fo[:, m, :], ps)

            def emit_out_dma(n, blk, fo):
                r0 = blk * BLK
                nc.sync.dma_start(
                    out=out_r[:, :, r0 + n * 4:r0 + n * 4 + 4, :]
                    .rearrange("k p r w -> p k (r w)"),
                    in_=fo)

            def emit_oproj(n, blk, aot_sb):
                fo = fop.tile([P, NCH, 512], dt.float32, tag="fo", name="fo")
                for m in range(NCH):
                    emit_oproj_m(n, m, aot_sb, fo, nc.scalar)
                emit_out_dma(n, blk, fo)

            def emit_v(r, xt_sb, vmix):
                psv = ps_pp.tile([P, 512], dt.float32, tag="pp", name="ps")
                for k in range(NCH):
                    nc.tensor.matmul(
                        psv,
                        lhsT=xt_sb[:, k, r * P:(r + 1) * P],
                        rhs=vw_sb[:, k, :],
                        start=(k == 0), stop=(k == NCH - 1))
                nc.vector.tensor_add(
                    out=vmix[:, r, :].rearrange(
                        "p (h e) -> p h e", e=65)[:, :, 0:64],
                    in0=psv.rearrange("p (h e) -> p h e", e=64),
                    in1=pv_sb.rearrange("p (h e) -> p h e", e=64))

            prev = None  # (ao_sb, aot_sb, blk) of previous block

            for blk in range(NBLK):
                r0 = blk * BLK
                qq = qq_ab[blk % 2]
                kk = kk_ab[blk % 2]

                # ---- load X^T block ----
                xt_sb = xtp.tile([P, NCH, TOK], dt.bfloat16, tag="xt")
                for k in range(NCH):
                    nc.sync.dma_start(out=xt_sb[:, k, :],
                                      in_=xt_r[k, :, r0:r0 + BLK, :]
                                      .rearrange("p r w -> p (r w)"))

                # ---- K projection + kk builds ----
                kt = ktp.tile([P, NCH, TOK], dt.bfloat16, tag="kt")
                for m in range(NCH):
                    for n in range(NTT):
                        ps = proj(kw_sb, m, n, xt_sb)
                        nc.vector.tensor_scalar_add(
                            kt[:, m, n * 512:(n + 1) * 512], ps,
                            kb_sb[:, m:m + 1])
                    # native-parity halves: k + gq*pq (SBUF-only, gpsimd)
                    nc.gpsimd.tensor_add(
                        out=kk[0:64, 0, m, :],
                        in0=kt[0:64, m, :], in1=pq_rep[0:64, m, :])
                    nc.gpsimd.tensor_add(
                        out=kk[64:128, 1, m, :],
                        in0=kt[64:128, m, :], in1=pq_rep[64:128, m, :])
                    # opposite-parity halves: raw k, partition-shifted
                    nc.gpsimd.dma_start(out=kk[64:128, 0, m, :],
                                        in_=kt[0:64, m, :])
                    nc.gpsimd.dma_start(out=kk[0:64, 1, m, :],
                                        in_=kt[64:128, m, :])

                # ---- deferred tail of previous block ----
                if prev is not None:
                    p_ao, p_aot, p_blk = prev
                    emit_tr(BLK - 1, p_ao, p_aot)
                    emit_oproj(0, p_blk, p_aot)
                    emit_oproj(1, p_blk, p_aot)

                # ---- Q projection + qq q-half builds ----
                qt = qtp.tile([P, NCH, TOK], dt.bfloat16, tag="qt")
                for m in range(NCH):
                    for n in range(NTT):
                        ps = proj(qw_sb, m, n, xt_sb)
                        if n == 0:
                            nc.scalar.activation(
                                qt[:, m, n * 512:(n + 1) * 512], ps,
                                AF.Identity, bias=qb_sb[:, m:m + 1])
                        else:
                            nc.vector.tensor_scalar_add(
                                qt[:, m, n * 512:(n + 1) * 512], ps,
                                qb_sb[:, m:m + 1])
                    nc.sync.dma_start(out=qq[0:64, 0, m, :],
                                        in_=qt[0:64, m, :])
                    nc.sync.dma_start(out=qq[64:128, 1, m, :],
                                        in_=qt[64:128, m, :])

                # ---- vmix ----
                vmix = vmixp.tile([P, BLK, NH * 65], dt.bfloat16, tag="vmix")
                nc.vector.memset(
                    vmix.rearrange("p r (h e) -> p r h e", e=65)
                    [:, :, :, 64:65], 1.0)

                ao_sb = aop.tile([P, BLK, C], dt.bfloat16, tag="ao")
                aot_sb = aotp.tile([P, NCH, TOK], dt.bfloat16, tag="aot")
                exp_all = expp.tile([P, BLK, 2, 512], dt.bfloat16, tag="exp")

                emit_v(0, xt_sb, vmix)

                # ---- rows ----
                for r in range(BLK):
                    # scores: one K=128 matmul per head
                    for half in range(2):
                        pss = ps_sc.tile([P, 512], dt.float32, tag="sc",
                                         name="pss")
                        for hh in range(4):
                            h = half * 4 + hh
                            nc.tensor.matmul(
                                pss[:, hh * P:(hh + 1) * P],
                                lhsT=kk[:, h % 2, h // 2,
                                        r * P:(r + 1) * P],
                                rhs=qq[:, h % 2, h // 2, r * P:(r + 1) * P],
                                start=True, stop=True)
                        nc.scalar.activation(exp_all[:, r, half, :], pss,
                                             AF.Exp)
                    if r < BLK - 1:
                        emit_v(r + 1, xt_sb, vmix)
                    if r >= 1:
                        emit_tr(r - 1, ao_sb, aot_sb)
                    # AV + normalize
                    for half in range(2):
                        psa = ps_av.tile([P, 4 * 65], dt.float32, tag="av",
                                         name="psa")
                        for hh in range(4):
                            h = half * 4 + hh
                            nc.tensor.matmul(
                                psa[:, hh * 65:(hh + 1) * 65],
                                lhsT=exp_all[:, r, half,
                                             hh * P:(hh + 1) * P],
                                rhs=vmix[:, r, h * 65:(h + 1) * 65],
                                start=True, stop=True)
                        psa_v = psa.rearrange("p (h e) -> p h e", e=65)
                        rv = small.tile([P, 4, 1], dt.float32, tag="rv",
                                        name="rv")
                        nc.vector.reciprocal(rv, psa_v[:, :, 64:65])
                        nc.vector.tensor_mul(
                            ao_sb[:, r, half * 256:(half + 1) * 256]
                            .rearrange("p (h e) -> p h e", e=64),
                            psa_v[:, :, 0:64],
                            rv[:, :, :].to_broadcast((P, 4, 64)))


                prev = (ao_sb, aot_sb, blk)

            # final tail
            p_ao, p_aot, p_blk = prev
            emit_tr(BLK - 1, p_ao, p_aot)
            emit_oproj(0, p_blk, p_aot)
            emit_oproj(1, p_blk, p_aot)

    nc.compile()
    return nc


def _get_nc():
    global _CACHED_NC
    if _CACHED_NC is None:
        _CACHED_NC = _build_nc()
    return _CACHED_NC


def kernel(x, q_w, q_b, k_w, k_b, v_w, v_b, o_w, o_b,
           pos_q, pos_k, pos_v, g_q, g_k, g_v1, g_v2):
    global LAST_RESULTS
    x = np.asarray(x, dtype=np.float32)
    q_w = np.asarray(q_w, dtype=np.float32)
    k_w = np.asarray(k_w, dtype=np.float32)
    v_w = np.asarray(v_w, dtype=np.float32)
    o_w = np.asarray(o_w, dtype=np.float32)
    q_b = np.asarray(q_b, dtype=np.float32)
    k_b = np.asarray(k_b, dtype=np.float32)
    v_b = np.asarray(v_b, dtype=np.float32)
    o_b = np.asarray(o_b, dtype=np.float32)
    pq = np.asarray(pos_q, dtype=np.float32)[0, :, :W, :]  # [NH, W, HD]
    pk = np.asarray(pos_k, dtype=np.float32)[0, :, :W, :]
    pv = np.asarray(pos_v, dtype=np.float32)[0, :, :W, :]
    gq = float(np.asarray(g_q).reshape(-1)[0])
    gk = float(np.asarray(g_k).reshape(-1)[0])
    gv1 = float(np.asarray(g_v1).reshape(-1)[0])
    gv2 = float(np.asarray(g_v2).reshape(-1)[0])

    scale = HD ** (-0.5)

    # chunk-major per core: [NCH, P, RPC, W] for 4KB-contiguous DMA runs
    xt_all = x.transpose(0, 2, 1, 3).reshape(ROWS, C, W).astype(BF16)
    q_wt = np.ascontiguousarray(q_w.T * scale).astype(BF16)
    k_wt = np.ascontiguousarray(k_w.T).astype(BF16)
    v_wt = np.ascontiguousarray(v_w.T * gv1).astype(BF16)
    o_wt = np.ascontiguousarray(o_w.T).astype(BF16)
    qb_s = (q_b * scale).astype(np.float32)
    kb_s = k_b.astype(np.float32)
    # pq^T chunk-major [NCH, P, W], replicated BLK times along tokens
    pqts = (gq * pq).transpose(0, 2, 1).reshape(NCH, P, W)
    pq_rep = np.ascontiguousarray(
        np.tile(pqts[:, :, None, :], (1, 1, BLK, 1)).reshape(NCH, P, TOK)
    ).astype(BF16)
    # qq constant halves: gk*scale*pk at the opposite parity of each head
    qq0 = np.zeros((P, 2, NCH, TOK), np.float32)
    pk_s = gk * scale * pk  # [NH, W, HD]
    for h in range(NH):
        par = h % 2
        opp = 1 - par
        blkv = np.tile(pk_s[h].T, (1, BLK))  # [HD, TOK]
        qq0[opp * HD:(opp + 1) * HD, par, h // 2, :] = blkv
    qq0 = qq0.astype(BF16)
    pvs = np.ascontiguousarray(
        gv2 * pv.transpose(1, 0, 2).reshape(W, C)
        + gv1 * v_b[None, :]).astype(BF16)
    ident = np.eye(P, dtype=np.float32).astype(BF16)

    shared = {
        "q_wt": q_wt, "k_wt": k_wt, "v_wt": v_wt, "o_wt": o_wt,
        "qb": qb_s, "kb": kb_s,
        "pq_rep": pq_rep, "qq0": qq0, "pvs": pvs, "ident": ident,
    }
    in_maps = []
    for c in range(NCORES):
        m = dict(shared)
        xs = xt_all[c * RPC:(c + 1) * RPC]  # [RPC, C, W]
        m["xt"] = np.ascontiguousarray(
            xs.reshape(RPC, NCH, P, W).transpose(1, 2, 0, 3))
        in_maps.append(m)

    nc = _get_nc()
    res = run_bass_kernel_spmd(nc, in_maps, core_ids=list(range(NCORES)))
    LAST_RESULTS = res

    out_all = np.concatenate(
        [res.results[c]["out_t"].transpose(2, 0, 1, 3).reshape(RPC, C, W)
         for c in range(NCORES)], axis=0)
    y = out_all.reshape(B, H, C, W).transpose(0, 2, 1, 3)
    y = y + o_b[None, :, None, None]
    return np.ascontiguousarray(y.astype(np.float32))


# revision 8
# speedup vs baseline: 1.0591x; 1.0591x over previous
"""Gated axial attention (width axis) Trainium2 Bass kernel.

Sharding: data-parallel over the fused B*H row axis (512 rows -> 64 rows
per core on 8 cores). Each core computes full attention for its rows;
no collectives. All matmuls run in bf16 with fp32 PSUM accumulation.

v2 design (vs baseline):
- Scores in ONE K=128 matmul per (head, row) instead of three K=64
  matmuls: contract [k + gq*pq ; k] (kk) against [q ; gk*s*pk] (qq),
  both stacked to 128 partitions per head.  kk native-parity halves are
  built by GPSIMD adds (k + pq_rep); opposite halves and the qq q-halves
  are built by SBUF->SBUF DMA from the projection outputs; qq pk-halves
  are constant (DMA'd once from DRAM).
- All score/AV matmul operands sit at full 128 partitions (no even/odd
  parity PSUM split) -> simple [128,512] score PSUM, double-buffered.
- Softmax normalize uses one broadcast tensor_mul per (row, half) with a
  stride-0 AP on the reciprocal denominators (replaces 8 tiny
  tensor_scalar ops per row).
- Per-row PE emission interleaves scores / V-proj(r+1) / transpose(r-1) /
  AV so the Exp ACT latency is hidden without double PSUM.
- Last-4-rows transposes + o-projection of block b are emitted inside
  block b+1's projection phase so the DVE backlog drains off-critical.
- Startup const DMAs spread across gpsimd/vector/scalar queues.

Scale folding (host side): 1/sqrt(hd)=0.125 into q_w/q_b; g_q into
pq_rep; g_k*0.125 into the qq pk-halves; g_v1 into v_w and v_b; g_v2
into pos_v; o_b added on host after gathering.
"""

import sys
import types

sys.path.insert(0, "/opt/trn_rl_repo")


def _install_ntff_shim():
    """Make bass_utils trace=True work under axon (BASS_TRACE=1)."""
    try:
        import antenv
    except ImportError:
        return
    if "antenv.axon_hooks" in sys.modules:
        return
    mod = types.ModuleType("antenv.axon_hooks")
    _hook = [None]

    def set_axon_ntff_profile_hook(h):
        _hook[0] = h

    def get_axon_ntff_profile_hook():
        if _hook[0] is None:
            try:
                if "/root/.axon_site" not in sys.path:
                    sys.path.insert(0, "/root/.axon_site")
                from trn_agent_boot.trn_boot import _ntff_profile_via_ctypes

                _hook[0] = _ntff_profile_via_ctypes("/opt/axon/libaxon_pjrt.so")
            except Exception:
                _hook[0] = None
        return _hook[0]

    mod.set_axon_ntff_profile_hook = set_axon_ntff_profile_hook
    mod.get_axon_ntff_profile_hook = get_axon_ntff_profile_hook
    sys.modules["antenv.axon_hooks"] = mod
    antenv.axon_hooks = mod


_install_ntff_shim()

import ml_dtypes  # noqa: E402
import numpy as np  # noqa: E402

import concourse.bass as bass  # noqa: E402
import concourse.tile as tile  # noqa: E402
from concourse import bacc, mybir  # noqa: E402
from concourse.bass_utils import run_bass_kernel_spmd  # noqa: E402

BF16 = ml_dtypes.bfloat16

B, C, H, W = 4, 512, 128, 128
NH, HD = 8, 64
NCORES = 8
ROWS = B * H  # 512 fused rows
RPC = ROWS // NCORES  # 64 rows per core
BLK = 8  # rows per block
NBLK = RPC // BLK
P = 128
NCH = C // P  # 4 channel chunks of 128
TOK = BLK * W  # tokens per block (1024)
NTT = TOK // 512  # 512-token tiles per block (2)

_CACHED_NC = None
LAST_RESULTS = None


def _build_nc():
    nc = bacc.Bacc("TRN2", target_bir_lowering=False, debug=False,
                   num_devices=NCORES)
    dt = mybir.dt

    xt = nc.dram_tensor("xt", [NCH, P, RPC, W], dt.bfloat16,
                        kind="ExternalInput")
    q_wt = nc.dram_tensor("q_wt", [C, C], dt.bfloat16, kind="ExternalInput")
    k_wt = nc.dram_tensor("k_wt", [C, C], dt.bfloat16, kind="ExternalInput")
    v_wt = nc.dram_tensor("v_wt", [C, C], dt.bfloat16, kind="ExternalInput")
    o_wt = nc.dram_tensor("o_wt", [C, C], dt.bfloat16, kind="ExternalInput")
    qb = nc.dram_tensor("qb", [C], dt.float32, kind="ExternalInput")
    kb = nc.dram_tensor("kb", [C], dt.float32, kind="ExternalInput")
    pq_rep_d = nc.dram_tensor("pq_rep", [NCH, P, TOK], dt.bfloat16,
                              kind="ExternalInput")
    qq0_d = nc.dram_tensor("qq0", [P, 2, NCH, TOK], dt.bfloat16,
                           kind="ExternalInput")
    pvs = nc.dram_tensor("pvs", [W, C], dt.bfloat16, kind="ExternalInput")
    ident = nc.dram_tensor("ident", [P, P], dt.bfloat16, kind="ExternalInput")
    out_t = nc.dram_tensor("out_t", [NCH, P, RPC, W], dt.float32,
                           kind="ExternalOutput")

    AF = mybir.ActivationFunctionType

    with tile.TileContext(nc) as tc:
        with (
            tc.tile_pool(name="const", bufs=1) as const,
            tc.tile_pool(name="xtp", bufs=2) as xtp,
            tc.tile_pool(name="qtp", bufs=1) as qtp,
            tc.tile_pool(name="ktp", bufs=1) as ktp,
            tc.tile_pool(name="vmixp", bufs=1) as vmixp,
            tc.tile_pool(name="expp", bufs=1) as expp,
            tc.tile_pool(name="aop", bufs=2) as aop,
            tc.tile_pool(name="aotp", bufs=2) as aotp,
            tc.tile_pool(name="small", bufs=4) as small,
            tc.tile_pool(name="fop", bufs=2) as fop,
            tc.tile_pool(name="ps_pp", bufs=2, space="PSUM") as ps_pp,
            tc.tile_pool(name="ps_sc", bufs=2, space="PSUM") as ps_sc,
            tc.tile_pool(name="ps_av", bufs=2, space="PSUM") as ps_av,
            tc.tile_pool(name="ps_tr", bufs=2, space="PSUM") as ps_tr,
        ):
            # ---- constants into SBUF (spread across DMA queues) ----
            def load_w(name, dram, eng):
                t = const.tile([P, NCH, C], dt.bfloat16, name=name)
                src = dram.ap().rearrange("(k p) c -> p k c", p=P)
                eng.dma_start(out=t, in_=src)
                return t

            # K-proj runs first: kw on gpsimd queue, first in line.
            kw_sb = load_w("kw_sb", k_wt, nc.gpsimd)
            qw_sb = load_w("qw_sb", q_wt, nc.gpsimd)
            vw_sb = load_w("vw_sb", v_wt, nc.scalar)
            ow_sb = load_w("ow_sb", o_wt, nc.scalar)

            pv_sb = const.tile([P, C], dt.bfloat16, name="pv_sb")
            nc.scalar.dma_start(out=pv_sb, in_=pvs.ap())
            id_sb = const.tile([P, P], dt.bfloat16, name="id_sb")
            nc.scalar.dma_start(out=id_sb, in_=ident.ap())
            qb_sb = const.tile([P, NCH], dt.float32, name="qb_sb")
            nc.scalar.dma_start(out=qb_sb,
                                in_=qb.ap().rearrange("(m p) -> p m", p=P))
            kb_sb = const.tile([P, NCH], dt.float32, name="kb_sb")
            nc.scalar.dma_start(out=kb_sb,
                                in_=kb.ap().rearrange("(m p) -> p m", p=P))
            pq_rep = const.tile([P, NCH, TOK], dt.bfloat16, name="pq_rep")
            nc.scalar.dma_start(out=pq_rep,
                                in_=pq_rep_d.ap().rearrange("k p t -> p k t"))
            # Persistent double-buffered qq/kk (manual A/B alternation).
            qq_ab = [const.tile([P, 2, NCH, TOK], dt.bfloat16, name="qqA"),
                     const.tile([P, 2, NCH, TOK], dt.bfloat16, name="qqB")]
            kk_ab = [const.tile([P, 2, NCH, TOK], dt.bfloat16, name="kkA"),
                     const.tile([P, 2, NCH, TOK], dt.bfloat16, name="kkB")]
            for t in qq_ab:
                nc.scalar.dma_start(out=t, in_=qq0_d.ap())

            xt_r = xt.ap()  # [NCH, P, RPC, W]
            out_r = out_t.ap()

            # ---- helpers ----
            def proj(wsb, m, n, xt_sb):
                ps = ps_pp.tile([P, 512], dt.float32, tag="pp", name="ps")
                for k in range(NCH):
                    nc.tensor.matmul(
                        ps,
                        lhsT=wsb[:, k, m * P:(m + 1) * P],
                        rhs=xt_sb[:, k, n * 512:(n + 1) * 512],
                        start=(k == 0), stop=(k == NCH - 1))
                return ps

            tr_state = {}  # parity -> pst tile awaiting its pair

            def emit_tr(r, ao_sb, aot_sb):
                # rows are transposed in pairs sharing one PSUM tile; the
                # copyback happens once per pair (on the odd row)
                if r % 2 == 0:
                    pst = ps_tr.tile([P, 2, NCH, P], dt.bfloat16, tag="tr",
                                     name="pst")
                    tr_state['pst'] = pst
                else:
                    pst = tr_state['pst']
                for ch in range(NCH):
                    nc.tensor.transpose(
                        pst[:, r % 2, ch, :],
                        ao_sb[:, r, ch * P:(ch + 1) * P], id_sb)
                if r % 2 == 1:
                    r0 = r - 1
                    nc.vector.tensor_copy(
                        aot_sb[:, :, r0 * P:(r0 + 2) * P]
                        .rearrange("p k (r w) -> p r k w", r=2),
                        pst.rearrange("p r k w -> p r k w"))

            def emit_oproj_m(n, m, aot_sb, fo, eng):
                ps = ps_pp.tile([P, 512], dt.float32, tag="pp", name="ps")
                for k in range(NCH):
                    nc.tensor.matmul(
                        ps,
                        lhsT=ow_sb[:, k, m * P:(m + 1) * P],
                        rhs=aot_sb[:, k, n * 512:(n + 1) * 512],
                        start=(k == 0), stop=(k == NCH - 1))
                if eng is nc.scalar:
                    eng.copy(fo[:, m, :], ps)
                else:
                    eng.tensor_copy(fo[:, m, :], ps)

            def emit_out_dma(n, blk, fo):
                r0 = blk * BLK
                nc.sync.dma_start(
                    out=out_r[:, :, r0 + n * 4:r0 + n * 4 + 4, :]
                    .rearrange("k p r w -> p k (r w)"),
                    in_=fo)

            def emit_oproj(n, blk, aot_sb):
                fo = fop.tile([P, NCH, 512], dt.float32, tag="fo", name="fo")
                for m in range(NCH):
                    emit_oproj_m(n, m, aot_sb, fo, nc.scalar)
                emit_out_dma(n, blk, fo)

            def emit_v(r, xt_sb, vmix):
                psv = ps_pp.tile([P, 512], dt.float32, tag="pp", name="ps")
                for k in range(NCH):
                    nc.tensor.matmul(
                        psv,
                        lhsT=xt_sb[:, k, r * P:(r + 1) * P],
                        rhs=vw_sb[:, k, :],
                        start=(k == 0), stop=(k == NCH - 1))
                nc.vector.tensor_add(
                    out=vmix[:, r, :].rearrange(
                        "p (h e) -> p h e", e=65)[:, :, 0:64],
                    in0=psv.rearrange("p (h e) -> p h e", e=64),
                    in1=pv_sb.rearrange("p (h e) -> p h e", e=64))

            prev = None  # (ao_sb, aot_sb, blk) of previous block

            for blk in range(NBLK):
                r0 = blk * BLK
                qq = qq_ab[blk % 2]
                kk = kk_ab[blk % 2]

                # ---- load X^T block ----
                xt_sb = xtp.tile([P, NCH, TOK], dt.bfloat16, tag="xt")
                for k in range(NCH):
                    nc.sync.dma_start(out=xt_sb[:, k, :],
                                      in_=xt_r[k, :, r0:r0 + BLK, :]
                                      .rearrange("p r w -> p (r w)"))

                # ---- K projection + kk builds ----
                kt = ktp.tile([P, NCH, TOK], dt.bfloat16, tag="kt")
                for m in range(NCH):
                    for n in range(NTT):
                        ps = proj(kw_sb, m, n, xt_sb)
                        nc.vector.tensor_scalar_add(
                            kt[:, m, n * 512:(n + 1) * 512], ps,
                            kb_sb[:, m:m + 1])
                    # native-parity halves: k + gq*pq (SBUF-only, gpsimd)
                    nc.gpsimd.tensor_add(
                        out=kk[0:64, 0, m, :],
                        in0=kt[0:64, m, :], in1=pq_rep[0:64, m, :])
                    nc.gpsimd.tensor_add(
                        out=kk[64:128, 1, m, :],
                        in0=kt[64:128, m, :], in1=pq_rep[64:128, m, :])
                    # opposite-parity halves: raw k, partition-shifted
                    nc.gpsimd.dma_start(out=kk[64:128, 0, m, :],
                                        in_=kt[0:64, m, :])
                    nc.gpsimd.dma_start(out=kk[0:64, 1, m, :],
                                        in_=kt[64:128, m, :])

                # ---- deferred tail of previous block ----
                if prev is not None:
                    p_ao, p_aot, p_blk = prev
                    emit_tr(BLK - 1, p_ao, p_aot)
                    emit_oproj(1, p_blk, p_aot)

                # ---- Q projection + qq q-half builds ----
                qt = qtp.tile([P, NCH, TOK], dt.bfloat16, tag="qt")
                for m in range(NCH):
                    for n in range(NTT):
                        ps = proj(qw_sb, m, n, xt_sb)
                        if n == 0:
                            nc.scalar.activation(
                                qt[:, m, n * 512:(n + 1) * 512], ps,
                                AF.Identity, bias=qb_sb[:, m:m + 1])
                        else:
                            nc.vector.tensor_scalar_add(
                                qt[:, m, n * 512:(n + 1) * 512], ps,
                                qb_sb[:, m:m + 1])
                    nc.sync.dma_start(out=qq[0:64, 0, m, :],
                                        in_=qt[0:64, m, :])
                    nc.sync.dma_start(out=qq[64:128, 1, m, :],
                                        in_=qt[64:128, m, :])

                # ---- vmix ----
                vmix = vmixp.tile([P, BLK, NH * 65], dt.bfloat16, tag="vmix")
                nc.vector.memset(
                    vmix.rearrange("p r (h e) -> p r h e", e=65)
                    [:, :, :, 64:65], 1.0)

                ao_sb = aop.tile([P, BLK, C], dt.bfloat16, tag="ao")
                aot_sb = aotp.tile([P, NCH, TOK], dt.bfloat16, tag="aot")
                exp_all = expp.tile([P, BLK, 2, 512], dt.bfloat16, tag="exp")

                emit_v(0, xt_sb, vmix)
                fo0 = fop.tile([P, NCH, 512], dt.float32, tag="fo",
                               name="fo")

                # ---- rows ----
                for r in range(BLK):
                    # scores: one K=128 matmul per head
                    for half in range(2):
                        pss = ps_sc.tile([P, 512], dt.float32, tag="sc",
                                         name="pss")
                        for hh in range(4):
                            h = half * 4 + hh
                            nc.tensor.matmul(
                                pss[:, hh * P:(hh + 1) * P],
                                lhsT=kk[:, h % 2, h // 2,
                                        r * P:(r + 1) * P],
                                rhs=qq[:, h % 2, h // 2, r * P:(r + 1) * P],
                                start=True, stop=True)
                        nc.scalar.activation(exp_all[:, r, half, :], pss,
                                             AF.Exp)
                    if r < BLK - 1:
                        emit_v(r + 1, xt_sb, vmix)
                    if r >= 1:
                        emit_tr(r - 1, ao_sb, aot_sb)
                    # AV + normalize
                    for half in range(2):
                        psa = ps_av.tile([P, 4 * 65], dt.float32, tag="av",
                                         name="psa")
                        for hh in range(4):
                            h = half * 4 + hh
                            nc.tensor.matmul(
                                psa[:, hh * 65:(hh + 1) * 65],
                                lhsT=exp_all[:, r, half,
                                             hh * P:(hh + 1) * P],
                                rhs=vmix[:, r, h * 65:(h + 1) * 65],
                                start=True, stop=True)
                        psa_v = psa.rearrange("p (h e) -> p h e", e=65)
                        rv = small.tile([P, 4, 1], dt.float32, tag="rv",
                                        name="rv")
                        nc.vector.reciprocal(rv, psa_v[:, :, 64:65])
                        nc.vector.tensor_mul(
                            ao_sb[:, r, half * 256:(half + 1) * 256]
                            .rearrange("p (h e) -> p h e", e=64),
                            psa_v[:, :, 0:64],
                            rv[:, :, :].to_broadcast((P, 4, 64)))
                    if r >= 4:
                        emit_oproj_m(0, r - 4, aot_sb, fo0, nc.scalar)
                emit_out_dma(0, blk, fo0)

                prev = (ao_sb, aot_sb, blk)

            # final tail
            p_ao, p_aot, p_blk = prev
            emit_tr(BLK - 1, p_ao, p_aot)
            emit_oproj(1, p_blk, p_aot)

    nc.compile()
    return nc


def _get_nc():
    global _CACHED_NC
    if _CACHED_NC is None:
        _CACHED_NC = _build_nc()
    return _CACHED_NC


def kernel(x, q_w, q_b, k_w, k_b, v_w, v_b, o_w, o_b,
           pos_q, pos_k, pos_v, g_q, g_k, g_v1, g_v2):
    global LAST_RESULTS
    x = np.asarray(x, dtype=np.float32)
    q_w = np.asarray(q_w, dtype=np.float32)
    k_w = np.asarray(k_w, dtype=np.float32)
    v_w = np.asarray(v_w, dtype=np.float32)
    o_w = np.asarray(o_w, dtype=np.float32)
    q_b = np.asarray(q_b, dtype=np.float32)
    k_b = np.asarray(k_b, dtype=np.float32)
    v_b = np.asarray(v_b, dtype=np.float32)
    o_b = np.asarray(o_b, dtype=np.float32)
    pq = np.asarray(pos_q, dtype=np.float32)[0, :, :W, :]  # [NH, W, HD]
    pk = np.asarray(pos_k, dtype=np.float32)[0, :, :W, :]
    pv = np.asarray(pos_v, dtype=np.float32)[0, :, :W, :]
    gq = float(np.asarray(g_q).reshape(-1)[0])
    gk = float(np.asarray(g_k).reshape(-1)[0])
    gv1 = float(np.asarray(g_v1).reshape(-1)[0])
    gv2 = float(np.asarray(g_v2).reshape(-1)[0])

    scale = HD ** (-0.5)

    # chunk-major per core: [NCH, P, RPC, W] for 4KB-contiguous DMA runs
    xt_all = x.transpose(0, 2, 1, 3).reshape(ROWS, C, W).astype(BF16)
    q_wt = np.ascontiguousarray(q_w.T * scale).astype(BF16)
    k_wt = np.ascontiguousarray(k_w.T).astype(BF16)
    v_wt = np.ascontiguousarray(v_w.T * gv1).astype(BF16)
    o_wt = np.ascontiguousarray(o_w.T).astype(BF16)
    qb_s = (q_b * scale).astype(np.float32)
    kb_s = k_b.astype(np.float32)
    # pq^T chunk-major [NCH, P, W], replicated BLK times along tokens
    pqts = (gq * pq).transpose(0, 2, 1).reshape(NCH, P, W)
    pq_rep = np.ascontiguousarray(
        np.tile(pqts[:, :, None, :], (1, 1, BLK, 1)).reshape(NCH, P, TOK)
    ).astype(BF16)
    # qq constant halves: gk*scale*pk at the opposite parity of each head
    qq0 = np.zeros((P, 2, NCH, TOK), np.float32)
    pk_s = gk * scale * pk  # [NH, W, HD]
    for h in range(NH):
        par = h % 2
        opp = 1 - par
        blkv = np.tile(pk_s[h].T, (1, BLK))  # [HD, TOK]
        qq0[opp * HD:(opp + 1) * HD, par, h // 2, :] = blkv
    qq0 = qq0.astype(BF16)
    pvs = np.ascontiguousarray(
        gv2 * pv.transpose(1, 0, 2).reshape(W, C)
        + gv1 * v_b[None, :]).astype(BF16)
    ident = np.eye(P, dtype=np.float32).astype(BF16)

    shared = {
        "q_wt": q_wt, "k_wt": k_wt, "v_wt": v_wt, "o_wt": o_wt,
        "qb": qb_s, "kb": kb_s,
        "pq_rep": pq_rep, "qq0": qq0, "pvs": pvs, "ident": ident,
    }
    in_maps = []
    for c in range(NCORES):
        m = dict(shared)
        xs = xt_all[c * RPC:(c + 1) * RPC]  # [RPC, C, W]
        m["xt"] = np.ascontiguousarray(
            xs.reshape(RPC, NCH, P, W).transpose(1, 2, 0, 3))
        in_maps.append(m)

    nc = _get_nc()
    res = run_bass_kernel_spmd(nc, in_maps, core_ids=list(range(NCORES)))
    LAST_RESULTS = res

    out_all = np.concatenate(
        [res.results[c]["out_t"].transpose(2, 0, 1, 3).reshape(RPC, C, W)
         for c in range(NCORES)], axis=0)
    y = out_all.reshape(B, H, C, W).transpose(0, 2, 1, 3)
    y = y + o_b[None, :, None, None]
    return np.ascontiguousarray(y.astype(np.float32))
